# revision 82
# baseline (speedup 1.0000x reference)
"""Trainium2 Bass kernel for a single pre-norm transformer block.

Reference math (B=4, T=2048, C=512, H=8, D=64, fp32):
    h  = LN(x; g1, b1) ; q,k,v = h @ Wq/Wk/Wv (per head)
    wei = softmax_over_QUERY_axis( causal_mask(q k^T / sqrt(C)) )
    x2  = x + concat_heads(wei @ v) @ Wo + bo
    out = x2 + relu(LN(x2; g2, b2) @ W1 + b1) @ W2 + b2

Sharding over 8 NeuronCores: pairs of cores per batch element
(core = 2*b + r). Each core computes LN1 + QKV for its batch,
attention for its 4 heads (h = 4r..4r+3), and the partial output
projection (contracting only its heads' features). A pairwise
ReduceScatter sums the partial projections and hands each core its
half of the tokens; the FFN is token-parallel (1024 tokens/core).

Optimization notes (245us -> 205us on the hardware-calibrated cost
model):
- all matmul operands are bf16 (weights converted host-side); rel err
  3.8e-3 vs the 2e-2 gate. fp8 fails numerically (>2.2e-2 everywhere).
- score tiles anchored at the causal boundary t0 = 128*i: no wasted
  columns are computed or exp'd. Narrow tiles (i >= 12) live in 1-bank
  PSUM slots so the wide slots are free for the next head's hoisted
  first scores (no WAR stall at head boundaries).
- the exp stream on ACT is the attention-phase bottleneck (~88us:
  17408 cols/head at 0.833ns + ~372ns/instr init+accum overhead);
  everything else is scheduled around keeping it dense. Exp and Sqrt
  live in different ACT table sets (1.3us per switch), so all LN sqrts
  are batched few-wide and ordered before/after the stream, with a
  data-dependent bias operand (zero_t/eps2) pinning them there.
- phase A is stage-pipelined per LN group with engine spreading
  (stats/aggr DVE, sqrt ACT, normalize DVE/Pool, hT copies DVE as
  2-tile doubles parked in idle wide-PSUM slots); pair-0 q/k follows
  each group so exps start ~20us in. v and pair-1 q/k are deferred
  into attention heads 0/1 where the exp stream has slack.
- proj/ReduceScatter/residual+LN2-stats run as hooks inside head 3's
  block boundaries; LN2 normalize+transpose is deferred past the last
  exp; collective traffic is batched into one DMA per chunk.
- GPSIMD (Pool) cannot touch PSUM on real HW (walrus rejects it even
  though the cost model accepts it), and AluOpType.divide does not
  codegen -- only SBUF-only mult/add-style Pool ops are used.
"""

import sys

sys.path.insert(0, "/opt/trn_rl_repo")

import ml_dtypes
import numpy as np

B, T, C, H, D = 4, 2048, 512, 8, 64
EPS = 1e-5
NCORES = 8
TH = T // 2  # tokens per core in the FFN phase
HPC = H // 2  # heads per core
NT = T // 128  # 16 token tiles per batch
NEG = -1e30

_CACHE: dict = {}


def _build_program(flags, sim=False):
    from contextlib import ExitStack

    import concourse.bacc as bacc
    import concourse.tile as tile
    from concourse import mybir
    from concourse.masks import make_identity

    has_bqkv, has_bo, has_b2 = flags
    f32 = mybir.dt.float32
    bf16 = mybir.dt.bfloat16
    AF = mybir.ActivationFunctionType
    ALU = mybir.AluOpType

    nc = bacc.Bacc(
        "TRN2", target_bir_lowering=False, debug=False,
        num_devices=1 if sim else NCORES,
    )

    x_e = nc.dram_tensor("x", [T, C], f32, kind="ExternalInput").ap()
    xh_e = nc.dram_tensor("xh", [TH, C], f32, kind="ExternalInput").ap()
    wq_e = nc.dram_tensor("wq", [C, HPC * D], bf16, kind="ExternalInput").ap()
    wk_e = nc.dram_tensor("wk", [C, HPC * D], bf16, kind="ExternalInput").ap()
    wv_e = nc.dram_tensor("wv", [C, HPC * D], bf16, kind="ExternalInput").ap()
    wo_e = nc.dram_tensor("wo", [128, 2, C], bf16, kind="ExternalInput").ap()
    w1_e = nc.dram_tensor("w1", [C, 4 * C], bf16, kind="ExternalInput").ap()
    b1_e = nc.dram_tensor("b1", [4 * C], f32, kind="ExternalInput").ap()
    w2_e = nc.dram_tensor("w2", [4 * C, C], bf16, kind="ExternalInput").ap()
    if has_bqkv:
        bq_e = nc.dram_tensor("bq", [1, HPC * D], bf16, kind="ExternalInput").ap()
        bk_e = nc.dram_tensor("bk", [1, HPC * D], bf16, kind="ExternalInput").ap()
        bv_e = nc.dram_tensor("bv", [1, HPC * D], bf16, kind="ExternalInput").ap()
    if has_bo:
        bo_e = nc.dram_tensor("bo", [C], f32, kind="ExternalInput").ap()
    if has_b2:
        b2_e = nc.dram_tensor("b2", [C], f32, kind="ExternalInput").ap()
    y_e = nc.dram_tensor("y", [TH, C], f32, kind="ExternalOutput").ap()

    NCH = 4  # collective chunks
    cc_in = [nc.dram_tensor(f"cc_in{k}", [T // NCH, C], bf16)
             for k in range(NCH)]
    cc_out = [nc.dram_tensor(f"cc_out{k}", [T // NCH // 2, C], bf16)
              for k in range(NCH)]

    with tile.TileContext(nc) as tc, ExitStack() as ctx:
        consts = ctx.enter_context(tc.tile_pool(name="consts", bufs=1))
        smalls = ctx.enter_context(tc.tile_pool(name="smalls", bufs=2))
        qk_pool = ctx.enter_context(tc.tile_pool(name="qk", bufs=1))
        vpool = ctx.enter_context(tc.tile_pool(name="vp", bufs=1))
        fw = ctx.enter_context(tc.tile_pool(name="fw", bufs=1))
        h2p = ctx.enter_context(tc.tile_pool(name="h2p", bufs=1))

        # ---- constants ----
        ident_b = consts.tile([128, 128], bf16)
        make_identity(nc, ident_b)
        mb_b = consts.tile([128, 128], bf16)
        nc.gpsimd.memset(mb_b[:], 0.0)
        nc.gpsimd.affine_select(
            out=mb_b[:], in_=mb_b[:], compare_op=ALU.is_ge, fill=NEG,
            base=0, pattern=[[1, 128]], channel_multiplier=-1,
        )
        eps_t = consts.tile([128, 1], f32)
        nc.vector.memset(eps_t[:], EPS)
        b1_sb = consts.tile([128, 16], f32)
        if has_bqkv:
            ones_sb = consts.tile([1, 512], bf16)
            nc.vector.memset(ones_sb[:], 1.0)
            bq_sb = consts.tile([1, HPC * D], bf16)
            nc.sync.dma_start(bq_sb[:], bq_e)
            bk_sb = consts.tile([1, HPC * D], bf16)
            nc.sync.dma_start(bk_sb[:], bk_e)
            bv_sb = consts.tile([1, HPC * D], bf16)
            nc.sync.dma_start(bv_sb[:], bv_e)
        if has_bo:
            import concourse.bass as bass

            bo_sb = consts.tile([128, C], f32)
            bo_b = bo_e[None, :]
            bo_bc = bass.AP(
                tensor=bo_b.tensor, offset=bo_b.offset,
                ap=[[0, 128], bo_b.ap[1]],
            )
            nc.sync.dma_start(bo_sb[:], bo_bc)
        if has_b2:
            import concourse.bass as bass

            b2_sb = consts.tile([128, C], f32)
            b2_b = b2_e[None, :]
            b2_bc = bass.AP(
                tensor=b2_b.tensor, offset=b2_b.offset,
                ap=[[0, 128], b2_b.ap[1]],
            )
            nc.sync.dma_start(b2_sb[:], b2_bc)

        # persistent activations / weights
        qT = qk_pool.tile([128, 2, T], bf16)  # [pair-head d, pair, t]
        kT = qk_pool.tile([128, 2, T], bf16)
        v_sb = vpool.tile([128, NT, HPC * D], bf16)  # [s in tile, tile, h*d]
        w1_sb = fw.tile([128, 4, 4 * C], bf16)
        w2_sb = fw.tile([128, 16, C], bf16)
        wo2_sb = fw.tile([128, 2, C], bf16)  # chunk p rows = heads 2p,2p+1
        xh_sb = fw.tile([128, 8, C], f32)  # this core's FFN-half of x
        h2T = h2p.tile([128, 4, TH], bf16)  # [c in chunk, chunk, t]
        # LN2 mean/var, split per half so the batched rstd for tiles 0-3
        # carries no false dependency on the later chunks' stats
        mvs_a = h2p.tile([128, 4, 2], f32)
        mvs_b = h2p.tile([128, 4, 2], f32)

        def layer_norm_stats(xm):
            """-> (rstd[128,1], mv[128,2]) for per-token LN over C."""
            stats = smalls.tile([128, 6], f32, tag="bnst")
            nc.vector.bn_stats(stats[:], xm)
            mv = smalls.tile([128, 2], f32, tag="bnag")
            nc.vector.bn_aggr(mv[:], stats[:])
            rstd = smalls.tile([128, 1], f32, tag="rstd")
            nc.scalar.activation(rstd[:], mv[:, 1:2], AF.Sqrt, bias=eps_t[:],
                                 scale=1.0)
            nc.vector.reciprocal(rstd[:], rstd[:])
            return rstd, mv

        # ============ Phases A+B (shared PSUM + deferred QKV) ============
        with ExitStack() as ab:
            # one PSUM pool: "sc" = score tiles (2 x 3 banks),
            # "sm" = all transient 512-col tiles (2 x 1 bank). 16KB exact.
            ps5 = ab.enter_context(tc.tile_pool(name="ps5", bufs=2,
                                                space="PSUM"))
            expp = ab.enter_context(tc.tile_pool(name="expp", bufs=1))
            attnp = ab.enter_context(tc.tile_pool(name="attnp", bufs=1))
            vsp = ab.enter_context(tc.tile_pool(name="vsp", bufs=1))
            zp = ab.enter_context(tc.tile_pool(name="zp", bufs=2))

            attn = [
                attnp.tile([128, T], bf16, tag=f"attnp{p}", name=f"attnp{p}")
                for p in range(2)
            ]

            def sm(name, dtype=f32, w=512):
                return ps5.tile([128, w], dtype, tag="sm", name=name)

            pre_scores = {}

            def emit_scores(h, i):
                p, u = h // 2, h % 2
                usl = slice(64 * u, 64 * u + 64)
                t0 = 128 * i
                W = T - t0
                Wm = min(W, 1536)
                if W <= 512:
                    # narrow tiles use the 1-bank slots; this also frees the
                    # wide "sc" slots early so the next head's first score
                    # tiles (hoisted) never wait on this head's last exps
                    ps = sm(f"sc_{h}_{i}")
                else:
                    ps = ps5.tile([128, 1536], f32, tag="sc",
                                  name=f"sc_{h}_{i}")
                nchunks = (Wm + 511) // 512
                for sb in range(nchunks):
                    w_ = min(512, Wm - 512 * sb)
                    nc.tensor.matmul(
                        ps[:, 512 * sb:512 * sb + w_],
                        lhsT=kT[usl, p, i * 128:(i + 1) * 128],
                        rhs=qT[usl, p, t0 + 512 * sb:t0 + 512 * sb + w_],
                        start=True, stop=(sb > 0),
                    )
                    if sb == 0:
                        # causal mask for the diagonal 128-block
                        nc.tensor.matmul(
                            ps[:, 0:128], lhsT=ident_b[:], rhs=mb_b[:],
                            start=False, stop=True, skip_group_check=True,
                        )
                ps2 = None
                if W > 1536:
                    ps2 = sm(f"sct_{h}_{i}")
                    nc.tensor.matmul(
                        ps2[:, 0:W - 1536],
                        lhsT=kT[usl, p, i * 128:(i + 1) * 128],
                        rhs=qT[usl, p, t0 + 1536:T],
                        start=True, stop=True,
                    )
                return ps, ps2

            def emit_qk1(p, tch, which, on_act):
                """one of q^T/k^T for pair p, 512-token chunk tch."""
                tsl = slice(tch * 512, (tch + 1) * 512)
                psl = slice(p * 128, (p + 1) * 128)
                wsb, dst = (wq_sb, qT) if which == 0 else (wk_sb, kT)
                qp = sm(f"qk_{p}_{tch}_{which}")
                for cc_ in range(4):
                    nc.tensor.matmul(
                        qp[:], lhsT=wsb[:, cc_, psl],
                        rhs=hT[:, cc_, tsl],
                        start=(cc_ == 0),
                        stop=(cc_ == 3 and not has_bqkv),
                    )
                if has_bqkv:
                    bsb = bq_sb if which == 0 else bk_sb
                    nc.tensor.matmul(
                        qp[:], lhsT=bsb[0:1, psl], rhs=ones_sb[0:1, :],
                        start=False, stop=True, skip_group_check=True,
                    )
                if on_act:
                    nc.scalar.copy(dst[:, p, tsl], qp[:])
                else:
                    nc.vector.tensor_copy(dst[:, p, tsl], qp[:])

            def emit_qk(p, tch, on_act):
                emit_qk1(p, tch, 0, on_act)
                emit_qk1(p, tch, 1, on_act)

            def emit_v(i):
                """v rows for token tile i (all 4 heads along free axis)."""
                vp_ = sm(f"v_{i}", w=256)
                for cc_ in range(4):
                    nc.tensor.matmul(
                        vp_[:], lhsT=hT[:, cc_, i * 128:(i + 1) * 128],
                        rhs=wv_sb[:, cc_, :],
                        start=(cc_ == 0),
                        stop=(cc_ == 3 and not has_bqkv),
                    )
                if has_bqkv:
                    nc.tensor.matmul(
                        vp_[:], lhsT=ones_sb[0:1, :128], rhs=bv_sb[0:1, :],
                        start=False, stop=True, skip_group_check=True,
                    )
                nc.vector.tensor_copy(v_sb[:, i, :], vp_[:])

            def do_x2_stats(m2, pt):
                """residual add + LN2 stats for one 128-token tile (no PE
                or table-switching ACT work -- those defer to phase C so
                the exp stream is never interrupted). Chunks 0/1 gate the
                first FFN half, so their adds take the faster DVE."""
                x2m = xh_sb[:, m2, :]
                eng = nc.vector if m2 < 4 else nc.gpsimd
                eng.tensor_tensor(x2m, x2m, pt, ALU.add)
                if has_bo:
                    nc.vector.tensor_tensor(x2m, x2m, bo_sb[:], ALU.add)
                stats = smalls.tile([128, 6], f32, tag="bnst")
                nc.vector.bn_stats(stats[:], x2m)
                mvx = mvs_a if m2 < 4 else mvs_b
                nc.vector.bn_aggr(mvx[:, m2 % 4, :], stats[:])

            def ln2_apply(m2, rstds8, ps_alloc):
                """deferred LN2 normalize + transpose for one m2-tile
                (the rstds come pre-batched so only one Sqrt hits the ACT
                table after the exp stream ends)."""
                x2m = xh_sb[:, m2, :]
                mvx = mvs_a if m2 < 4 else mvs_b
                rstd = rstds8[:, m2 % 4:m2 % 4 + 1]
                hm = smalls.tile([128, C], bf16, tag="h2m", bufs=4)
                eng = nc.gpsimd if m2 % 2 else nc.vector
                eng.tensor_scalar(
                    hm[:], x2m, mvx[:, m2 % 4, 0:1], rstd,
                    ALU.subtract, ALU.mult,
                )
                tp = ps_alloc(m2)
                for cc_ in range(4):
                    nc.tensor.transpose(
                        tp[:, cc_ * 128:(cc_ + 1) * 128],
                        hm[:, cc_ * 128:(cc_ + 1) * 128],
                        ident_b[:],
                    )
                nc.scalar.copy(h2T[:, :, m2 * 128:(m2 + 1) * 128], tp[:])

            def proj_chunk(k):
                """partial output projection + ReduceScatter for chunk k,
                then residual+stats for the 2 m2-tiles it unblocks."""
                pjc = smalls.tile([128, 4, 512], bf16, tag="pj")
                for mm_ in range(NT // NCH):
                    m = k * (NT // NCH) + mm_
                    pp = sm(f"pp_{m}")
                    for p_ in range(2):
                        nc.tensor.matmul(
                            pp[:],
                            lhsT=attn[p_][:, m * 128:(m + 1) * 128],
                            rhs=wo2_sb[:, p_, :],
                            start=(p_ == 0), stop=(p_ == 1),
                        )
                    if k == 3:
                        # post-exp: the ACT engine is idle here
                        nc.scalar.copy(pjc[:, mm_, :], pp[:])
                    else:
                        nc.vector.tensor_copy(pjc[:, mm_, :], pp[:])
                nc.sync.dma_start(
                    cc_in[k].ap().rearrange("(mm p) c -> p mm c", p=128),
                    pjc[:],
                )
                if sim:
                    nc.sync.dma_start(cc_out[k].ap(),
                                      cc_in[k].ap()[:T // NCH // 2, :])
                else:
                    nc.gpsimd.collective_compute(
                        "ReduceScatter",
                        ALU.add,
                        replica_groups=[[0, 1], [2, 3], [4, 5], [6, 7]],
                        ins=[cc_in[k].ap()],
                        outs=[cc_out[k].ap()],
                    )
                ptc = smalls.tile([128, 2, 512], bf16, tag="pr")
                with tc.high_priority():
                    nc.sync.dma_start(
                        ptc[:],
                        cc_out[k].ap().rearrange("(mm p) c -> p mm c",
                                                 p=128),
                    )
                    do_x2_stats(2 * k, ptc[:, 0, :])
                    do_x2_stats(2 * k + 1, ptc[:, 1, :])

            def head(h, extra_pe=None, block_hook=None):
                """scores -> exp -> AV for head h; extra_pe(i) interleaves
                independent PE work, block_hook(j) runs after AV block j
                (used by the last head to pipeline proj/RS/LN2)."""
                p, u = h // 2, h % 2
                par = min(h % 2, 1)
                usl = slice(64 * u, 64 * u + 64)
                z = zp.tile([128, NT], f32, tag="z")
                head.last_z = z
                zr = zp.tile([128, NT], f32, tag="zr")
                vs = vsp.tile([128, NT, D], bf16, tag="vs", name=f"vs{h}")
                exps = []
                for i in range(NT):
                    if extra_pe is not None:
                        extra_pe(i)
                    t0 = 128 * i
                    W = T - t0
                    if (h, i) in pre_scores:
                        ps, ps2 = pre_scores.pop((h, i))
                    else:
                        ps, ps2 = emit_scores(h, i)
                    et = expp.tile([128, W], bf16, tag=f"e{i}",
                                   bufs=2 if i < 2 else 1,
                                   name=f"exp_{h}_{i}")
                    exps.append(et)
                    if ps2 is None:
                        nc.scalar.activation(
                            et[:, 0:W], ps[:, 0:W],
                            AF.Exp, bias=zero_t[:], scale=1.0,
                            accum_out=z[:, i:i + 1],
                        )
                    else:
                        zpt = zp.tile([128, 1], f32, tag="zpart")
                        nc.scalar.activation(
                            et[:, 0:1536], ps[:, 0:1536],
                            AF.Exp, bias=zero_t[:], scale=1.0, accum_out=zpt[:],
                        )
                        zpt2 = zp.tile([128, 1], f32, tag="zpart2")
                        nc.scalar.activation(
                            et[:, 1536:W], ps2[:, 0:W - 1536],
                            AF.Exp, bias=zero_t[:], scale=1.0, accum_out=zpt2[:],
                        )
                        nc.vector.tensor_tensor(
                            z[:, i:i + 1], zpt[:], zpt2[:], ALU.add
                        )
                    # row i complete: 1/Z and scaled v rows for this tile
                    nc.vector.reciprocal(zr[:, i:i + 1], z[:, i:i + 1])
                    nc.gpsimd.tensor_scalar(
                        vs[:, i, :], v_sb[:, i, h * D:(h + 1) * D],
                        zr[:, i:i + 1], None, ALU.mult,
                    )
                    if i == NT - 1 and h < HPC - 1:
                        # hoist the next head's first score tiles so the
                        # exp stream never waits on this head's last AV
                        pre_scores[(h + 1, 0)] = emit_scores(h + 1, 0)
                        pre_scores[(h + 1, 1)] = emit_scores(h + 1, 1)
                    # AV for t-block j unlocks once rows 0..4j+3 are done
                    if i % 4 == 3:
                        j = i // 4
                        av = sm(f"av_{h}_{j}")
                        for ii in range(4 * j + 4):
                            off = 128 * ii - 512 * j
                            if off <= 0:
                                nc.tensor.matmul(
                                    av[usl, :],
                                    lhsT=vs[:, ii, :],
                                    rhs=exps[ii][:, -off:-off + 512],
                                    start=(ii == 0), stop=(ii == 4 * j + 3),
                                )
                            else:
                                nc.tensor.matmul(
                                    av[usl, off:],
                                    lhsT=vs[:, ii, :],
                                    rhs=exps[ii][:, 0:512 - off],
                                    start=False, stop=(ii == 4 * j + 3),
                                    skip_group_check=True,
                                )
                        nc.vector.tensor_copy(
                            attn[p][usl, j * 512:(j + 1) * 512], av[usl, :]
                        )
                        if block_hook is not None:
                            block_hook(j)

            # ---- phase A proper (x -> LN1 -> hT -> pair-0 q/k) ----
            with ExitStack() as a2:
                wq_pool = a2.enter_context(tc.tile_pool(name="wqkv", bufs=1))
                wq_sb = wq_pool.tile([128, 4, HPC * D], bf16)
                wk_sb = wq_pool.tile([128, 4, HPC * D], bf16)
                wv_sb = wq_pool.tile([128, 4, HPC * D], bf16)
                hT_pool = a2.enter_context(tc.tile_pool(name="hT", bufs=1))
                hT = hT_pool.tile([128, 4, T], bf16)  # [c in chunk, chunk, t]

                with ExitStack() as xs:
                    xpool = xs.enter_context(
                        tc.tile_pool(name="xp", bufs=6))
                    hpool = xs.enter_context(
                        tc.tile_pool(name="hn", bufs=4))
                    lnp = xs.enter_context(
                        tc.tile_pool(name="lnp", bufs=8))
                    x_r = x_e.rearrange("(n p) c -> p n c", p=128)
                    xts = []
                    # x tiles 0/1 lead the weight DMAs so LN starts early
                    for xc in range(8):
                        xt = xpool.tile([128, 2, C], f32, tag="x",
                                        name=f"x_{xc}")
                        if xc == 0:
                            nc.sync.dma_start(xt[:, 0:1, :],
                                              x_r[:, 0:1, :])
                            nc.sync.dma_start(xt[:, 1:2, :],
                                              x_r[:, 1:2, :])
                        else:
                            nc.sync.dma_start(xt[:],
                                              x_r[:, 2 * xc:2 * xc + 2, :])
                        xts.append(xt)
                        if xc == 7:
                            nc.sync.dma_start(
                                b1_sb[:],
                                b1_e.rearrange("(n p) -> p n", p=128))
                        if xc == 3:
                            for wsb, wee in ((wq_sb, wq_e), (wk_sb, wk_e),
                                             (wv_sb, wv_e)):
                                nc.sync.dma_start(
                                    wsb[:],
                                    wee.rearrange("(o p) d -> p o d", p=128))
                    # stage-major over 8-tile super-chunks: each engine gets
                    # long independent bursts, no cross-engine round-trips
                    rstd8_last = None
                    ln_groups = [(0, 1), (2, 3), (4, 5, 6, 7),
                                 (8, 9, 10, 11), (12, 13, 14, 15)]
                    qk_after = {1: 0, 2: 1, 3: 2, 4: 3}
                    for gi, ms in enumerate(ln_groups):
                        gn = len(ms)
                        amv8f = lnp.tile([128, 4, 2], f32, tag="amv",
                                         bufs=3, name="amv8")
                        rstd8f = lnp.tile([128, 4], f32, tag="arstd",
                                          bufs=3, name="rstd8")
                        amv8 = amv8f[:, 0:gn, :]
                        rstd8 = rstd8f[:, 0:gn]
                        rstd8_last = rstd8
                        for mm, m in enumerate(ms):
                            xm = xts[m // 2][:, m % 2, :]
                            stats = smalls.tile([128, 6], f32, tag="bnst")
                            nc.vector.bn_stats(stats[:], xm)
                            nc.vector.bn_aggr(amv8[:, mm, :], stats[:])
                        # ONE Sqrt + reciprocal per LN group: few ACT
                        # table interactions without a front barrier
                        nc.scalar.activation(rstd8[:], amv8[:, :, 1],
                                             AF.Sqrt, bias=eps_t[:],
                                             scale=1.0)
                        nc.vector.reciprocal(rstd8f[:, 0:gn], rstd8f[:, 0:gn])
                        tp = None
                        for mm, m in enumerate(ms):
                            xm = xts[m // 2][:, m % 2, :]
                            mv = amv8[:, mm, :]
                            rstd = rstd8[:, mm:mm + 1]
                            hm = hpool.tile([128, C], bf16, tag="hm")
                            eng = nc.vector if m % 2 == 0 else nc.gpsimd
                            eng.tensor_scalar(
                                hm[:], xm, mv[0:128, 0:1], rstd,
                                ALU.subtract, ALU.mult,
                            )
                            # transpose pairs of tiles into one 2-tile psum
                            # (parked in an idle "sc" slot) so the hT copies
                            # are half as many, twice as wide
                            if m % 2 == 0:
                                tp = ps5.tile([128, 1024], bf16, tag="sc",
                                              name=f"tp_{m}")
                            half = (m % 2) * 512
                            for cc_ in range(4):
                                nc.tensor.transpose(
                                    tp[:, half + cc_ * 128:
                                       half + (cc_ + 1) * 128],
                                    hm[:, cc_ * 128:(cc_ + 1) * 128],
                                    ident_b[:],
                                )
                            if m % 2 == 1:
                                import concourse.bass as bass

                                dst = hT[:, :, (m - 1) * 128:(m + 1) * 128]
                                dst4 = dst.rearrange("p cc (mm t) -> p cc mm t",
                                                     mm=2)
                                a4 = tp[:].rearrange(
                                    "p (mm cc t) -> p mm cc t", mm=2, cc=4)
                                src4 = bass.AP(
                                    tensor=a4.tensor, offset=a4.offset,
                                    ap=[a4.ap[0], a4.ap[2], a4.ap[1],
                                        a4.ap[3]],
                                )
                                nc.vector.tensor_copy(dst4, src4)
                        if gi in qk_after:
                            emit_qk(0, qk_after[gi], on_act=True)

                    # exp bias operand, data-dependent on the final LN1
                    # rstd batch: guarantees every Sqrt precedes the first
                    # Exp so each ACT table is loaded exactly once
                    zero_t = consts.tile([128, 1], f32)
                    nc.vector.tensor_scalar(
                        zero_t[:], rstd8_last[:, 3:4], 0.0, 0.0,
                        ALU.mult, ALU.mult,
                    )

                # weights for proj/FFN load behind the x tiles in the
                # DMA queue; all arrive long before they are needed
                nc.sync.dma_start(
                    wo2_sb[:], wo_e)
                nc.sync.dma_start(
                    xh_sb[:], xh_e.rearrange("(n p) c -> p n c", p=128))
                nc.sync.dma_start(
                    w1_sb[:], w1_e.rearrange("(o p) n -> p o n", p=128))
                nc.sync.dma_start(
                    w2_sb[:], w2_e.rearrange("(o p) c -> p o c", p=128))

                # deferred work rides inside the first two heads, placed
                # at i where the exp stream is ahead of the score matmuls
                qk1_sched_h0 = {1: (1, 0, 0), 2: (1, 0, 1), 3: (1, 1, 0)}
                qk1_sched_h1 = {1: (1, 1, 1), 2: (1, 2, 0), 3: (1, 2, 1),
                                4: (1, 3, 0), 5: (1, 3, 1)}

                def extra_h0(i):
                    emit_v(i)
                    if i in qk1_sched_h0:
                        p_, t_, w_ = qk1_sched_h0[i]
                        emit_qk1(p_, t_, w_, on_act=False)

                def extra_h1(i):
                    if i in qk1_sched_h1:
                        p_, t_, w_ = qk1_sched_h1[i]
                        emit_qk1(p_, t_, w_, on_act=False)

                head(0, extra_pe=extra_h0)
                head(1, extra_pe=extra_h1)
            head(2)
            head(3, block_hook=proj_chunk)
            # batched LN2 rstd: ONE Sqrt for all 8 tiles, with its bias
            # input data-dependent on the last head's z so the scheduler
            # cannot hoist it into the exp stream (Sqrt and Exp live in
            # different ACT table sets -- each switch costs a 1.3us load)
            eps2 = h2p.tile([128, 1], f32)
            nc.gpsimd.tensor_scalar(
                eps2[:], head.last_z[:, NT - 1:NT], 0.0, EPS,
                ALU.mult, ALU.add,
            )
            rstds_a = h2p.tile([128, 4], f32)
            rstds_b = h2p.tile([128, 4], f32)
            # high priority: the FFN can only start once these land, and
            # every data dependency (eps2 <- last z) is already explicit
            with tc.high_priority():
                nc.scalar.activation(rstds_a[:], mvs_a[:, :, 1], AF.Sqrt,
                                     bias=eps2[:], scale=1.0)
                nc.vector.reciprocal(rstds_a[:], rstds_a[:])
                for m2 in range(4):
                    ln2_apply(m2, rstds_a,
                              lambda m: sm(f"tp2_{m}", dtype=bf16))
            # eps3 depends on rstds_a so the scheduler cannot order this
            # Sqrt (which waits on the late k2/k3 hook stats) ahead of the
            # FFN-gating one in the in-order ACT queue
            eps3 = h2p.tile([128, 1], f32)
            nc.gpsimd.tensor_scalar(
                eps3[:], rstds_a[:, 0:1], 0.0, EPS, ALU.mult, ALU.add,
            )
            nc.scalar.activation(rstds_b[:], mvs_b[:, :, 1], AF.Sqrt,
                                 bias=eps3[:], scale=1.0)
            nc.vector.reciprocal(rstds_b[:], rstds_b[:])

        # ================= Phase C: LN2 apply + FFN =================
        with ExitStack() as cs:
            relup = cs.enter_context(tc.tile_pool(name="relup", bufs=2))
            ps1 = cs.enter_context(
                tc.tile_pool(name="ps1", bufs=4, space="PSUM"))
            ps2p = cs.enter_context(
                tc.tile_pool(name="ps2p", bufs=2, space="PSUM"))
            psT2 = cs.enter_context(
                tc.tile_pool(name="psT2", bufs=2, space="PSUM"))

            for m2 in range(4, 8):
                ln2_apply(
                    m2, rstds_b,
                    lambda m: psT2.tile([128, 512], bf16, tag="tp2",
                                        name=f"tp2_{m}"))
            for tb in range(2):
                relu = relup.tile([128, 16, 512], bf16, tag="relu")
                for nn in range(16):
                    fp = ps1.tile([128, 512], f32, tag="fp")
                    for cc_ in range(4):
                        nc.tensor.matmul(
                            fp[:],
                            lhsT=w1_sb[:, cc_, nn * 128:(nn + 1) * 128],
                            rhs=h2T[:, cc_, tb * 512:(tb + 1) * 512],
                            start=(cc_ == 0), stop=(cc_ == 3),
                        )
                    # relu(psum + b1[n]) on ACT (bias is per-partition)
                    nc.scalar.activation(
                        relu[:, nn, :], fp[:], AF.Relu,
                        bias=b1_sb[:, nn:nn + 1], scale=1.0,
                    )
                for mloc in range(4):
                    m2 = tb * 4 + mloc
                    f2 = ps2p.tile([128, 512], f32, tag="f2")
                    for nn in range(16):
                        nc.tensor.matmul(
                            f2[:],
                            lhsT=relu[:, nn, mloc * 128:(mloc + 1) * 128],
                            rhs=w2_sb[:, nn, :],
                            start=(nn == 0), stop=(nn == 15),
                        )
                    yt = smalls.tile([128, 512], f32, tag="yt")
                    nc.vector.tensor_tensor(yt[:], f2[:],
                                            xh_sb[:, m2, :], ALU.add)
                    if has_b2:
                        nc.vector.tensor_tensor(yt[:], yt[:], b2_sb[:],
                                                ALU.add)
                    nc.sync.dma_start(y_e[m2 * 128:(m2 + 1) * 128, :], yt[:])

    nc.compile()
    return nc


def _make_runner(nc):
    """Build a cached jitted SPMD callable (adapted from
    bass2jax.run_bass_via_pjrt, so repeat timing calls skip re-tracing)."""
    import jax
    import numpy as np
    from jax.experimental.shard_map import shard_map
    from jax.sharding import Mesh, PartitionSpec

    from concourse import bass2jax, mybir

    bass2jax.install_neuronx_cc_hook()
    assert nc.dbg_addr is None
    partition_name = (
        nc.partition_id_tensor.name if nc.partition_id_tensor else None
    )

    in_names, out_names, out_avals, zero_shapes = [], [], [], []
    for alloc in nc.m.functions[0].allocations:
        if not isinstance(alloc, mybir.MemoryLocationSet):
            continue
        name = alloc.memorylocations[0].name
        if alloc.kind == "ExternalInput":
            if name != partition_name:
                in_names.append(name)
        elif alloc.kind == "ExternalOutput":
            out_names.append(name)
            shape = tuple(alloc.tensor_shape)
            dtype = mybir.dt.np(alloc.dtype)
            out_avals.append(jax.core.ShapedArray(shape, dtype))
            zero_shapes.append((shape, dtype))
    n_params = len(in_names)
    n_outs = len(out_avals)
    all_names = in_names + out_names
    if partition_name is not None:
        all_names = all_names + [partition_name]

    def _body(*args):
        operands = list(args)
        if partition_name is not None:
            operands.append(bass2jax.partition_id_tensor())
        outs = bass2jax._bass_exec_p.bind(
            *operands,
            out_avals=tuple(out_avals),
            in_names=tuple(all_names),
            out_names=tuple(out_names),
            lowering_input_output_aliases=(),
            sim_require_finite=True,
            sim_require_nnan=True,
            nc=nc,
        )
        return tuple(outs)

    devices = jax.devices()[:NCORES]
    mesh = Mesh(np.asarray(devices), ("core",))
    donate = tuple(range(n_params, n_params + n_outs))
    sharded = jax.jit(
        shard_map(
            _body,
            mesh=mesh,
            in_specs=(PartitionSpec("core"),) * (n_params + n_outs),
            out_specs=(PartitionSpec("core"),) * n_outs,
            check_rep=False,
        ),
        donate_argnums=donate,
        keep_unused=True,
    )

    def stage(in_maps):
        concat = [
            np.concatenate(
                [np.ascontiguousarray(m[name]) for m in in_maps], axis=0
            )
            for name in in_names
        ]
        dev_inputs = [jax.device_put(a) for a in concat]
        for a in dev_inputs:
            a.block_until_ready()
        return dev_inputs

    def stage_zeros():
        zeros = [
            jax.device_put(np.zeros((NCORES * s[0],) + tuple(s[1:]), d))
            for (s, d) in zero_shapes
        ]
        for z in zeros:
            z.block_until_ready()
        return zeros

    def execute(dev_inputs, dev_zeros):
        outs = sharded(*dev_inputs, *dev_zeros)
        for o in outs:
            o.block_until_ready()
        return outs

    def run(in_maps, dev_inputs=None):
        """Returns (per_core_outputs, dev_inputs_for_reuse)."""
        if dev_inputs is None:
            dev_inputs = stage(in_maps)
        outs = execute(dev_inputs, stage_zeros())
        outs = [np.asarray(o) for o in outs]
        per_core = []
        for c in range(NCORES):
            d = {}
            for i, name in enumerate(out_names):
                rows = zero_shapes[i][0][0]
                d[name] = outs[i][c * rows:(c + 1) * rows]
            per_core.append(d)
        return per_core, dev_inputs

    def sharded_call(dev_inputs, dev_zeros):
        return sharded(*dev_inputs, *dev_zeros)

    run.stage = stage
    run.stage_zeros = stage_zeros
    run.execute = execute
    run.sharded_call = sharded_call
    return run


def _shard_inputs(inputs):
    bfd = ml_dtypes.bfloat16
    x = np.asarray(inputs["x"], np.float32)
    Wq = np.asarray(inputs["Wq"], np.float32)
    Wk = np.asarray(inputs["Wk"], np.float32)
    Wv = np.asarray(inputs["Wv"], np.float32)
    Wo = np.asarray(inputs["Wo"], np.float32)
    bo = np.asarray(inputs["bo"], np.float32)
    W1 = np.asarray(inputs["W1"], np.float32)
    b1 = np.asarray(inputs["b1"], np.float32)
    W2 = np.asarray(inputs["W2"], np.float32)
    b2 = np.asarray(inputs["b2"], np.float32)
    g1 = np.asarray(inputs["g1"], np.float32)
    beta1 = np.asarray(inputs["beta1"], np.float32)
    g2 = np.asarray(inputs["g2"], np.float32)
    beta2 = np.asarray(inputs["beta2"], np.float32)

    scale = C ** -0.5
    # fold LN1 affine into the QKV weights (and the score scale into Wq)
    Wq_f = g1[None, :, None] * Wq * scale  # [H, C, D]
    Wk_f = g1[None, :, None] * Wk
    Wv_f = g1[None, :, None] * Wv
    bq_f = np.einsum("c,hcd->hd", beta1, Wq_f)  # [H, D]
    bk_f = np.einsum("c,hcd->hd", beta1, Wk_f)
    bv_f = np.einsum("c,hcd->hd", beta1, Wv_f)
    W1_f = g2[:, None] * W1
    b1_f = b1 + beta2 @ W1

    has_bqkv = bool(
        np.any(bq_f != 0) or np.any(bk_f != 0) or np.any(bv_f != 0)
    )
    has_bo = bool(np.any(bo != 0))
    has_b2 = bool(np.any(b2 != 0))
    flags = (has_bqkv, has_bo, has_b2)

    in_maps = []
    for c in range(NCORES):
        b, r = c // 2, c % 2
        hs = slice(HPC * r, HPC * (r + 1))
        m = {
            "x": np.ascontiguousarray(x[b]),
            "xh": np.ascontiguousarray(np.concatenate([
                x[b, k * 512 + r * 256:k * 512 + (r + 1) * 256]
                for k in range(4)
            ])),
            "wq": np.ascontiguousarray(
                Wq_f[hs].transpose(1, 0, 2).reshape(C, HPC * D)
            ).astype(bfd),
            "wk": np.ascontiguousarray(
                Wk_f[hs].transpose(1, 0, 2).reshape(C, HPC * D)
            ).astype(bfd),
            "wv": np.ascontiguousarray(
                Wv_f[hs].transpose(1, 0, 2).reshape(C, HPC * D)
            ).astype(bfd),
            # Wo rows pair-chunked: [hd-in-chunk, pair, c]
            "wo": np.ascontiguousarray(
                Wo[HPC * D * r:HPC * D * (r + 1)]
                .reshape(2, 128, C).transpose(1, 0, 2)
            ).astype(bfd),
            "w1": W1_f.astype(bfd),
            "b1": b1_f,
            "w2": W2.astype(bfd),
        }
        if has_bqkv:
            m["bq"] = bq_f[hs].reshape(1, HPC * D).astype(bfd)
            m["bk"] = bk_f[hs].reshape(1, HPC * D).astype(bfd)
            m["bv"] = bv_f[hs].reshape(1, HPC * D).astype(bfd)
        if has_bo:
            m["bo"] = bo
        if has_b2:
            m["b2"] = b2
        in_maps.append(m)
    return in_maps, flags


def _get_runner(flags):
    key = ("runner", flags)
    if key not in _CACHE:
        nc = _build_program(flags)
        _CACHE[key] = _make_runner(nc)
    return _CACHE[key]


def kernel(**inputs) -> np.ndarray:
    in_maps, flags = _shard_inputs(inputs)
    run = _get_runner(flags)
    per_core, dev_inputs = run(in_maps)
    _CACHE["last"] = (run, in_maps, dev_inputs)
    out = np.empty((B, T, C), np.float32)
    for c in range(NCORES):
        b, r = c // 2, c % 2
        y = per_core[c]["y"]
        for k in range(4):
            lo = k * 512 + r * 256
            out[b, lo:lo + 256] = y[k * 256:(k + 1) * 256]
    return out


def bench_pipelined(n=10):
    """Dispatch n executions back-to-back (async), return avg seconds/call
    for the last n-1 (first call absorbs queueing)."""
    import time

    run, in_maps, dev_inputs = _CACHE["last"]
    zsets = [run.stage_zeros() for _ in range(n)]
    # warm
    run.execute(dev_inputs, zsets[0])
    t0 = time.perf_counter()
    outs = []
    for i in range(1, n):
        outs.append(run.sharded_call(dev_inputs, zsets[i]))
    for os_ in outs:
        for o in os_:
            o.block_until_ready()
    t1 = time.perf_counter()
    return (t1 - t0) / (n - 1)


def timed_rerun():
    """Re-run the last kernel() invocation with device-resident inputs
    and pre-staged output buffers; returns wall seconds of execute only."""
    import time

    run, in_maps, dev_inputs = _CACHE["last"]
    dev_zeros = run.stage_zeros()
    t0 = time.perf_counter()
    run.execute(dev_inputs, dev_zeros)
    return time.perf_counter() - t0
